# revision 10
# baseline (speedup 1.0000x reference)
"""MedianBlur 3x3 (zero-padded) over (16, 3, 512, 512) fp32 on 8 NeuronCores.

Strategy
--------
Pure data parallel: batch dim 16 -> 2 per core; each core processes
6 images (2 batches x 3 channels) of 512x512.

Host side pads each image to 514x514 with zeros, so the device kernel
needs no boundary special-casing: the median of a 3x3 window of the
padded image (windows centered at padded rows/cols 1..512) equals the
reference's zero-padded median exactly.

Device layout: the 6 images are processed in 3 passes of 2 images.
Within a pass, each image occupies 64 partitions; partition p holds
K=8 output rows (10 padded rows with halo, each 514 floats) in its
free dimension, so BOTH the vertical and the horizontal 3-tap window
reads are free-dim offsets -- no transposes, no cross-partition
traffic.

Median-of-9 as a separable min/max network (exact, 18 tensor_tensor
ops per pass):
  vertical sort3 of each column  -> lo (L), mid (M), hi (Hh)
  median9 = med3( max3_h(L), med3_h(M), min3_h(Hh) )

Engine split: 13 ops on VectorE, 5 ops (the hi-chain pairwise mins and
mid-chain pairwise min/max) on GPSIMD, which runs concurrently.

Sync-wait hardware limit: walrus codegen allows a SINGLE sync-wait per
instruction (TensorTensor and HWDGE DMA alike). The kernel is
structured so no instruction ever needs two semaphores:
 * X tiles are DMA-written and get a fresh slot per pass (bufs=3) --
   no WAW-on-DMA waits; loads+stores total 6 <= 8 HWDGE queues so no
   queue-reuse waits.
 * Each SBUF tile is written by exactly one engine (X/PA, PVn, PVx,
   Hh, Mm by DVE; PC, PMn, PMx by GPSIMD), so a writer never needs
   both a DMA sem and a cross-engine sem.
 * The output stages into Hh (DVE-owned, dead after the GPSIMD
   pairwise mins), and PA reuses the X tile (dead after the vertical
   stage), so the store's WAR lands on an instruction that needs no
   other wait.
"""

import os
from contextlib import ExitStack

import numpy as np

import concourse.bacc as bacc
import concourse.bass as bass
import concourse.mybir as mybir
import concourse.tile as tile
from concourse.bass_utils import run_bass_kernel_spmd

FP32 = mybir.dt.float32
MIN = mybir.AluOpType.min
MAX = mybir.AluOpType.max

N_CORES = 8
B, C, H, W = 16, 3, 512, 512
IMGS = (B // N_CORES) * C  # images per core = 6
HP, WP = H + 2, W + 2      # zero-padded image
K = 8                      # output rows per partition
XROWS = K + 2              # input rows per partition (halo)
PIMG = H // K              # partitions per image = 64
PASS_IMGS = 128 // PIMG    # images per pass = 2
NPASS = IMGS // PASS_IMGS  # passes = 3

# GPSIMD offload is disabled: this walrus version cannot encode the
# TensorTensor opcode on the Pool engine (ISA check fails at codegen).
OFFLOAD = os.environ.get("MEDIAN_OFFLOAD", "0") == "1"

_cache = {}


def _build(offload: bool):
    # Bacc (not raw Bass): its generate_event_semaphores pass splits
    # multi-wait instructions, which TRN2 hardware cannot encode.
    nc = bacc.Bacc(
        "TRN2", target_bir_lowering=False, debug=False, num_devices=N_CORES
    )
    xp = nc.declare_dram_parameter("xp", [IMGS, HP, WP], FP32, isOutput=False)
    y = nc.declare_dram_parameter("y", [IMGS, H, W], FP32, isOutput=True)

    with ExitStack() as ctx:
        tc = ctx.enter_context(tile.TileContext(nc))
        px = ctx.enter_context(tc.tile_pool(name="px", bufs=NPASS))
        pt = ctx.enter_context(tc.tile_pool(name="pt", bufs=1))

        V = nc.vector
        G = nc.gpsimd if offload else nc.vector

        for ps in range(NPASS):
            X = px.tile([128, XROWS * WP], FP32, tag="X")
            nc.sync.dma_start(
                out=X[:],
                in_=bass.AP(
                    xp,
                    ps * PASS_IMGS * HP * WP,
                    [[HP * WP, PASS_IMGS], [K * WP, PIMG], [1, XROWS * WP]],
                ),
            )
            X3 = X.rearrange("p (r c) -> p r c", c=WP)  # [128, 10, 514]

            # DVE-owned tiles
            PVn = pt.tile([128, K * WP], FP32, tag="PVn")
            PVx = pt.tile([128, K * WP], FP32, tag="PVx")
            Hh = pt.tile([128, K * WP], FP32, tag="Hh")
            Mm = pt.tile([128, K * WP], FP32, tag="Mm")
            # GPSIMD-owned tiles
            PC = pt.tile([128, K * WP], FP32, tag="PC")
            PMn = pt.tile([128, K * WP], FP32, tag="PMn")
            PMx = pt.tile([128, K * WP], FP32, tag="PMx")

            r3 = lambda t: t.rearrange("p (r c) -> p r c", c=WP)
            PVn3, PVx3, Hh3, Mm3 = r3(PVn), r3(PVx), r3(Hh), r3(Mm)
            PC3, PMn3, PMx3 = r3(PC), r3(PMn), r3(PMx)
            # PA lives in the X tile (X is dead after the vertical stage)
            PA3 = X3[:, 0:K, :]

            # ---- vertical sort3 (per column), pairwise-shared ----
            V.tensor_tensor(PVn3, X3[:, 0:K, :], X3[:, 1 : K + 1, :], op=MIN)
            V.tensor_tensor(PVx3, X3[:, 0:K, :], X3[:, 1 : K + 1, :], op=MAX)
            # hi = max(pv_max, x+2)
            V.tensor_tensor(Hh3, PVx3, X3[:, 2 : K + 2, :], op=MAX)
            # T = min(pv_max, x+2)   (in place into PVx)
            V.tensor_tensor(PVx3, PVx3, X3[:, 2 : K + 2, :], op=MIN)
            # mid = max(pv_min, T)
            V.tensor_tensor(Mm3, PVn3, PVx3, op=MAX)
            # lo = min(pv_min, x+2)  (in place into PVn; X dead now)
            V.tensor_tensor(PVn3, PVn3, X3[:, 2 : K + 2, :], op=MIN)
            L3 = PVn3

            # ---- horizontal merge ----
            # C = min3_h(Hh) on GPSIMD
            G.tensor_tensor(PC3[:, :, 0:513], Hh3[:, :, 0:513], Hh3[:, :, 1:514], op=MIN)
            G.tensor_tensor(PC3[:, :, 0:512], PC3[:, :, 0:512], Hh3[:, :, 2:514], op=MIN)
            # mid pairwise on GPSIMD
            G.tensor_tensor(PMn3[:, :, 0:513], Mm3[:, :, 0:513], Mm3[:, :, 1:514], op=MIN)
            G.tensor_tensor(PMx3[:, :, 0:513], Mm3[:, :, 0:513], Mm3[:, :, 1:514], op=MAX)
            G.tensor_tensor(PMx3[:, :, 0:512], PMx3[:, :, 0:512], Mm3[:, :, 2:514], op=MIN)
            # A = max3_h(L) -> PA (in the dead X tile)
            V.tensor_tensor(PA3[:, :, 0:513], L3[:, :, 0:513], L3[:, :, 1:514], op=MAX)
            V.tensor_tensor(PA3[:, :, 0:512], PA3[:, :, 0:512], L3[:, :, 2:514], op=MAX)
            # B = max(PMn, TB)
            V.tensor_tensor(PMn3[:, :, 0:512], PMn3[:, :, 0:512], PMx3[:, :, 0:512], op=MAX)
            # med3(A, B, C): U = min(A,B) -> PMx; V2 = max(A,B) -> PA;
            # W2 = min(V2,C) -> PA; out = max(U,W2) -> Hh (dead, DVE-owned)
            V.tensor_tensor(PMx3[:, :, 0:512], PA3[:, :, 0:512], PMn3[:, :, 0:512], op=MIN)
            V.tensor_tensor(PA3[:, :, 0:512], PA3[:, :, 0:512], PMn3[:, :, 0:512], op=MAX)
            V.tensor_tensor(PA3[:, :, 0:512], PA3[:, :, 0:512], PC3[:, :, 0:512], op=MIN)
            V.tensor_tensor(Hh3[:, :, 0:512], PMx3[:, :, 0:512], PA3[:, :, 0:512], op=MAX)

            nc.sync.dma_start(
                out=bass.AP(
                    y,
                    ps * PASS_IMGS * H * W,
                    [[H * W, PASS_IMGS], [K * W, PIMG], [1, K * W]],
                ),
                in_=Hh3[:, :, 0:512],
            )
    nc.finalize()
    return nc


LAST_EXEC_TIME_NS = None
LAST_TRACE = None


def run(x: np.ndarray, trace: bool = False, offload: bool | None = None):
    """x: (16,3,512,512) fp32 -> (16,3,512,512) fp32 median-blurred."""
    global LAST_EXEC_TIME_NS, LAST_TRACE
    if offload is None:
        offload = OFFLOAD
    assert x.shape == (B, C, H, W), x.shape
    x = np.ascontiguousarray(x, dtype=np.float32)

    key = ("v3", offload)
    if key not in _cache:
        _cache[key] = _build(offload)
    nc = _cache[key]

    xpad = np.pad(x, ((0, 0), (0, 0), (1, 1), (1, 1)))
    shards = xpad.reshape(N_CORES, IMGS, HP, WP)
    in_maps = [{"xp": shards[c]} for c in range(N_CORES)]

    res = run_bass_kernel_spmd(nc, in_maps, list(range(N_CORES)), trace=trace)
    LAST_EXEC_TIME_NS = res.exec_time_ns
    LAST_TRACE = res.instructions_and_trace
    out = np.stack([res.results[c]["y"] for c in range(N_CORES)])
    return np.ascontiguousarray(out.reshape(B, C, H, W))


def kernel(x: np.ndarray) -> np.ndarray:
    return run(x, trace=False)


# revision 13
# speedup vs baseline: 1.0701x; 1.0701x over previous
"""MedianBlur 3x3 (zero-padded) over (16, 3, 512, 512) fp32 on 8 NeuronCores.

Strategy
--------
Pure data parallel: batch dim 16 -> 2 per core; each core processes
6 images (2 batches x 3 channels) of 512x512.

Host side pads each image to 514x514 with zeros, so the device kernel
needs no boundary special-casing: the median of a 3x3 window of the
padded image (windows centered at padded rows/cols 1..512) equals the
reference's zero-padded median exactly.

Device layout: the 6 images are processed in 3 passes of 2 images.
Within a pass, each image occupies 64 partitions; partition p holds
K=8 output rows (10 padded rows with halo, each 514 floats) in its
free dimension, so BOTH the vertical and the horizontal 3-tap window
reads are free-dim offsets -- no transposes, no cross-partition
traffic.

Median-of-9 as a separable min/max network (exact, 18 tensor_tensor
ops per pass):
  vertical sort3 of each column  -> lo (L), mid (M), hi (Hh)
  median9 = med3( max3_h(L), med3_h(M), min3_h(Hh) )

Engine split: 13 ops on VectorE, 5 ops (the hi-chain pairwise mins and
mid-chain pairwise min/max) on GPSIMD, which runs concurrently.

Sync-wait hardware limit: walrus codegen allows a SINGLE sync-wait per
instruction (TensorTensor and HWDGE DMA alike). The kernel is
structured so no instruction ever needs two semaphores:
 * X tiles are DMA-written and get a fresh slot per pass (bufs=3) --
   no WAW-on-DMA waits; loads+stores total 6 <= 8 HWDGE queues so no
   queue-reuse waits.
 * Each SBUF tile is written by exactly one engine (X/PA, PVn, PVx,
   Hh, Mm by DVE; PC, PMn, PMx by GPSIMD), so a writer never needs
   both a DMA sem and a cross-engine sem.
 * The output stages into Hh (DVE-owned, dead after the GPSIMD
   pairwise mins), and PA reuses the X tile (dead after the vertical
   stage), so the store's WAR lands on an instruction that needs no
   other wait.
"""

import os
from contextlib import ExitStack

import numpy as np

import concourse.bacc as bacc
import concourse.bass as bass
import concourse.mybir as mybir
import concourse.tile as tile
from concourse.bass_utils import run_bass_kernel_spmd

FP32 = mybir.dt.float32
MIN = mybir.AluOpType.min
MAX = mybir.AluOpType.max

N_CORES = 8
B, C, H, W = 16, 3, 512, 512
IMGS = (B // N_CORES) * C  # images per core = 6
HP, WP = H + 2, W + 2      # zero-padded image
K = 8                      # output rows per partition
XROWS = K + 2              # input rows per partition (halo)
PIMG = H // K              # partitions per image = 64
PASS_IMGS = 128 // PIMG    # images per pass = 2
NPASS = IMGS // PASS_IMGS  # passes = 3

# GPSIMD offload is disabled: this walrus version cannot encode the
# TensorTensor opcode on the Pool engine (ISA check fails at codegen).
OFFLOAD = os.environ.get("MEDIAN_OFFLOAD", "0") == "1"

_cache = {}


def _build(offload: bool):
    # Bacc (not raw Bass): its generate_event_semaphores pass splits
    # multi-wait instructions, which TRN2 hardware cannot encode.
    nc = bacc.Bacc(
        "TRN2", target_bir_lowering=False, debug=False, num_devices=N_CORES
    )
    xp = nc.declare_dram_parameter("xp", [IMGS, HP, WP], FP32, isOutput=False)
    y = nc.declare_dram_parameter("y", [IMGS, H, W], FP32, isOutput=True)

    with ExitStack() as ctx:
        tc = ctx.enter_context(tile.TileContext(nc))
        px = ctx.enter_context(tc.tile_pool(name="px", bufs=NPASS))
        pt = ctx.enter_context(tc.tile_pool(name="pt", bufs=1))

        V = nc.vector
        G = nc.gpsimd if offload else nc.vector

        # Issue ALL input loads up front, split into 16-partition chunks so
        # they spread across the 8 HWDGE queues (a single big DMA runs on
        # one queue at ~50 GB/s = 52 us/pass, serializing the kernel).
        # Loads go on the sync engine; stores on the scalar engine's HWDGE
        # stream so a store blocked on compute never delays later loads.
        LOAD_CHUNK = 16  # partitions per load DMA
        Xs = []
        for ps in range(NPASS):
            X = px.tile([128, XROWS * WP], FP32, tag="X")
            Xs.append(X)
            for p0 in range(0, 128, LOAD_CHUNK):
                img = ps * PASS_IMGS + p0 // PIMG
                row0 = (p0 % PIMG) * K
                nc.sync.dma_start(
                    out=X[p0 : p0 + LOAD_CHUNK, :],
                    in_=bass.AP(
                        xp,
                        img * HP * WP + row0 * WP,
                        [[K * WP, LOAD_CHUNK], [1, XROWS * WP]],
                    ),
                )

        for ps in range(NPASS):
            X = Xs[ps]
            X3 = X.rearrange("p (r c) -> p r c", c=WP)  # [128, 10, 514]

            # DVE-owned tiles
            PVn = pt.tile([128, K * WP], FP32, tag="PVn")
            PVx = pt.tile([128, K * WP], FP32, tag="PVx")
            Hh = pt.tile([128, K * WP], FP32, tag="Hh")
            Mm = pt.tile([128, K * WP], FP32, tag="Mm")
            # GPSIMD-owned tiles
            PC = pt.tile([128, K * WP], FP32, tag="PC")
            PMn = pt.tile([128, K * WP], FP32, tag="PMn")
            PMx = pt.tile([128, K * WP], FP32, tag="PMx")

            r3 = lambda t: t.rearrange("p (r c) -> p r c", c=WP)
            PVn3, PVx3, Hh3, Mm3 = r3(PVn), r3(PVx), r3(Hh), r3(Mm)
            PC3, PMn3, PMx3 = r3(PC), r3(PMn), r3(PMx)
            # PA lives in the X tile (X is dead after the vertical stage)
            PA3 = X3[:, 0:K, :]

            # ---- vertical sort3 (per column), pairwise-shared ----
            V.tensor_tensor(PVn3, X3[:, 0:K, :], X3[:, 1 : K + 1, :], op=MIN)
            V.tensor_tensor(PVx3, X3[:, 0:K, :], X3[:, 1 : K + 1, :], op=MAX)
            # hi = max(pv_max, x+2)
            V.tensor_tensor(Hh3, PVx3, X3[:, 2 : K + 2, :], op=MAX)
            # T = min(pv_max, x+2)   (in place into PVx)
            V.tensor_tensor(PVx3, PVx3, X3[:, 2 : K + 2, :], op=MIN)
            # mid = max(pv_min, T)
            V.tensor_tensor(Mm3, PVn3, PVx3, op=MAX)
            # lo = min(pv_min, x+2)  (in place into PVn; X dead now)
            V.tensor_tensor(PVn3, PVn3, X3[:, 2 : K + 2, :], op=MIN)
            L3 = PVn3

            # ---- horizontal merge ----
            # C = min3_h(Hh) on GPSIMD
            G.tensor_tensor(PC3[:, :, 0:513], Hh3[:, :, 0:513], Hh3[:, :, 1:514], op=MIN)
            G.tensor_tensor(PC3[:, :, 0:512], PC3[:, :, 0:512], Hh3[:, :, 2:514], op=MIN)
            # mid pairwise on GPSIMD
            G.tensor_tensor(PMn3[:, :, 0:513], Mm3[:, :, 0:513], Mm3[:, :, 1:514], op=MIN)
            G.tensor_tensor(PMx3[:, :, 0:513], Mm3[:, :, 0:513], Mm3[:, :, 1:514], op=MAX)
            G.tensor_tensor(PMx3[:, :, 0:512], PMx3[:, :, 0:512], Mm3[:, :, 2:514], op=MIN)
            # A = max3_h(L) -> PA (in the dead X tile)
            V.tensor_tensor(PA3[:, :, 0:513], L3[:, :, 0:513], L3[:, :, 1:514], op=MAX)
            V.tensor_tensor(PA3[:, :, 0:512], PA3[:, :, 0:512], L3[:, :, 2:514], op=MAX)
            # B = max(PMn, TB)
            V.tensor_tensor(PMn3[:, :, 0:512], PMn3[:, :, 0:512], PMx3[:, :, 0:512], op=MAX)
            # med3(A, B, C): U = min(A,B) -> PMx; V2 = max(A,B) -> PA;
            # W2 = min(V2,C) -> PA; out = max(U,W2) -> Hh (dead, DVE-owned)
            V.tensor_tensor(PMx3[:, :, 0:512], PA3[:, :, 0:512], PMn3[:, :, 0:512], op=MIN)
            V.tensor_tensor(PA3[:, :, 0:512], PA3[:, :, 0:512], PMn3[:, :, 0:512], op=MAX)
            V.tensor_tensor(PA3[:, :, 0:512], PA3[:, :, 0:512], PC3[:, :, 0:512], op=MIN)
            V.tensor_tensor(Hh3[:, :, 0:512], PMx3[:, :, 0:512], PA3[:, :, 0:512], op=MAX)

            STORE_CHUNK = 32  # partitions per store DMA
            for p0 in range(0, 128, STORE_CHUNK):
                img = ps * PASS_IMGS + p0 // PIMG
                row0 = (p0 % PIMG) * K
                nc.scalar.dma_start(
                    out=bass.AP(
                        y,
                        img * H * W + row0 * W,
                        [[K * W, STORE_CHUNK], [1, K * W]],
                    ),
                    in_=Hh3[p0 : p0 + STORE_CHUNK, :, 0:512],
                )
    nc.finalize()
    return nc


LAST_EXEC_TIME_NS = None
LAST_TRACE = None


def run(x: np.ndarray, trace: bool = False, offload: bool | None = None):
    """x: (16,3,512,512) fp32 -> (16,3,512,512) fp32 median-blurred."""
    global LAST_EXEC_TIME_NS, LAST_TRACE
    if offload is None:
        offload = OFFLOAD
    assert x.shape == (B, C, H, W), x.shape
    x = np.ascontiguousarray(x, dtype=np.float32)

    key = ("v4", offload)
    if key not in _cache:
        _cache[key] = _build(offload)
    nc = _cache[key]

    xpad = np.pad(x, ((0, 0), (0, 0), (1, 1), (1, 1)))
    shards = xpad.reshape(N_CORES, IMGS, HP, WP)
    in_maps = [{"xp": shards[c]} for c in range(N_CORES)]

    res = run_bass_kernel_spmd(nc, in_maps, list(range(N_CORES)), trace=trace)
    LAST_EXEC_TIME_NS = res.exec_time_ns
    LAST_TRACE = res.instructions_and_trace
    out = np.stack([res.results[c]["y"] for c in range(N_CORES)])
    return np.ascontiguousarray(out.reshape(B, C, H, W))


def kernel(x: np.ndarray) -> np.ndarray:
    return run(x, trace=False)


# revision 16
# speedup vs baseline: 1.1455x; 1.0705x over previous
"""MedianBlur 3x3 (zero-padded) over (16, 3, 512, 512) fp32 on 8 NeuronCores.

Strategy
--------
Pure data parallel: batch dim 16 -> 2 per core; each core processes
6 images (2 batches x 3 channels) of 512x512.

Host side pads each image to 514x514 with zeros, so the device kernel
needs no boundary special-casing: the median of a 3x3 window of the
padded image (windows centered at padded rows/cols 1..512) equals the
reference's zero-padded median exactly.

Device layout: the 6 images are processed in 3 passes of 2 images.
Within a pass, each image occupies 64 partitions; partition p holds
K=8 output rows (10 padded rows with halo, each 514 floats) in its
free dimension, so BOTH the vertical and the horizontal 3-tap window
reads are free-dim offsets -- no transposes, no cross-partition
traffic.

Median-of-9 as a separable min/max network (exact, 18 tensor_tensor
ops per pass):
  vertical sort3 of each column  -> lo (L), mid (M), hi (Hh)
  median9 = med3( max3_h(L), med3_h(M), min3_h(Hh) )

All 18 ops run on VectorE (fp32 tensor_tensor = 1 elem/lane/cycle; the
other engines cannot do 2-input elementwise min/max on this toolchain:
walrus rejects TensorTensor on Pool, ScalarE is unary-only). The
min/max network is the whole compute cost; buffers are reused
aggressively (5 SBUF tiles total) so the OUT staging tile (Hh) can be
double-buffered and stores overlap the next pass.

DMA: each HWDGE engine (sync, scalar) owns ONE ~100 GB/s hardware
queue. All loads are issued up front (X has a fresh slot per pass);
pass 0's load and the last pass's store are split across both engines
to shorten the exposed head/tail. Multi-wait instructions are
legalized by Bacc's generate_event_semaphores (TRN2 instructions
encode at most one sync-wait).
"""

import os
from contextlib import ExitStack

import numpy as np

import concourse.bacc as bacc
import concourse.bass as bass
import concourse.mybir as mybir
import concourse.tile as tile
from concourse.bass_utils import run_bass_kernel_spmd

FP32 = mybir.dt.float32
MIN = mybir.AluOpType.min
MAX = mybir.AluOpType.max

N_CORES = 8
B, C, H, W = 16, 3, 512, 512
IMGS = (B // N_CORES) * C  # images per core = 6
HP, WP = H + 2, W + 2      # zero-padded image
K = 8                      # output rows per partition
XROWS = K + 2              # input rows per partition (halo)
PIMG = H // K              # partitions per image = 64
PASS_IMGS = 128 // PIMG    # images per pass = 2
NPASS = IMGS // PASS_IMGS  # passes = 3

# GPSIMD offload is disabled: this walrus version cannot encode the
# TensorTensor opcode on the Pool engine (ISA check fails at codegen).
OFFLOAD = os.environ.get("MEDIAN_OFFLOAD", "0") == "1"

_cache = {}


def _build(offload: bool):
    # Bacc (not raw Bass): its generate_event_semaphores pass splits
    # multi-wait instructions, which TRN2 hardware cannot encode.
    nc = bacc.Bacc(
        "TRN2", target_bir_lowering=False, debug=False, num_devices=N_CORES
    )
    xp = nc.declare_dram_parameter("xp", [IMGS, HP, WP], FP32, isOutput=False)
    y = nc.declare_dram_parameter("y", [IMGS, H, W], FP32, isOutput=True)

    with ExitStack() as ctx:
        tc = ctx.enter_context(tile.TileContext(nc))
        px = ctx.enter_context(tc.tile_pool(name="px", bufs=NPASS))
        ph = ctx.enter_context(tc.tile_pool(name="ph", bufs=2))
        pt = ctx.enter_context(tc.tile_pool(name="pt", bufs=1))

        V = nc.vector

        # Issue ALL input loads up front. Each HWDGE engine owns ONE
        # hardware queue (~100 GB/s), so parallelism comes from using both
        # engines (sync + scalar), not from chunk count. Pass 0's load is
        # split across both so compute starts ~14us in; later loads stream
        # on sync while stores use scalar.
        LOAD_CHUNK = 16  # partitions per load DMA (keeps the queue fed)
        Xs = []
        for ps in range(NPASS):
            X = px.tile([128, XROWS * WP], FP32, tag="X")
            Xs.append(X)
            for ci, p0 in enumerate(range(0, 128, LOAD_CHUNK)):
                img = ps * PASS_IMGS + p0 // PIMG
                row0 = (p0 % PIMG) * K
                eng = nc.scalar if (ps == 0 and ci % 2 == 1) else nc.sync
                eng.dma_start(
                    out=X[p0 : p0 + LOAD_CHUNK, :],
                    in_=bass.AP(
                        xp,
                        img * HP * WP + row0 * WP,
                        [[K * WP, LOAD_CHUNK], [1, XROWS * WP]],
                    ),
                )

        for ps in range(NPASS):
            X = Xs[ps]
            X3 = X.rearrange("p (r c) -> p r c", c=WP)  # [128, 10, 514]

            PVn = pt.tile([128, K * WP], FP32, tag="PVn")
            PVx = pt.tile([128, K * WP], FP32, tag="PVx")
            Hh = ph.tile([128, K * WP], FP32, tag="Hh")  # bufs=2: store overlap
            Mm = pt.tile([128, K * WP], FP32, tag="Mm")

            r3 = lambda t: t.rearrange("p (r c) -> p r c", c=WP)
            PVn3, PVx3, Hh3, Mm3 = r3(PVn), r3(PVx), r3(Hh), r3(Mm)
            # PA lives in the X tile (X is dead after the vertical stage)
            PA3 = X3[:, 0:K, :]

            # ---- vertical sort3 (per column), pairwise-shared ----
            V.tensor_tensor(PVn3, X3[:, 0:K, :], X3[:, 1 : K + 1, :], op=MIN)
            V.tensor_tensor(PVx3, X3[:, 0:K, :], X3[:, 1 : K + 1, :], op=MAX)
            # hi = max(pv_max, x+2)
            V.tensor_tensor(Hh3, PVx3, X3[:, 2 : K + 2, :], op=MAX)
            # T = min(pv_max, x+2)   (in place)
            V.tensor_tensor(PVx3, PVx3, X3[:, 2 : K + 2, :], op=MIN)
            # mid = max(pv_min, T)
            V.tensor_tensor(Mm3, PVn3, PVx3, op=MAX)
            # lo = min(pv_min, x+2)  (in place; X dead now)
            V.tensor_tensor(PVn3, PVn3, X3[:, 2 : K + 2, :], op=MIN)
            L3 = PVn3

            # ---- horizontal merge (buffers cycle: every tile all-DVE) ----
            # A = max3_h(L) -> PA (in the dead X tile)
            V.tensor_tensor(PA3[:, :, 0:513], L3[:, :, 0:513], L3[:, :, 1:514], op=MAX)
            V.tensor_tensor(PA3[:, :, 0:512], PA3[:, :, 0:512], L3[:, :, 2:514], op=MAX)
            # C = min3_h(Hh) -> PVx (T dead)
            V.tensor_tensor(PVx3[:, :, 0:513], Hh3[:, :, 0:513], Hh3[:, :, 1:514], op=MIN)
            V.tensor_tensor(PVx3[:, :, 0:512], PVx3[:, :, 0:512], Hh3[:, :, 2:514], op=MIN)
            # mid pairwise: PMn -> PVn (L dead), PMx -> Hh (hi dead)
            V.tensor_tensor(PVn3[:, :, 0:513], Mm3[:, :, 0:513], Mm3[:, :, 1:514], op=MIN)
            V.tensor_tensor(Hh3[:, :, 0:513], Mm3[:, :, 0:513], Mm3[:, :, 1:514], op=MAX)
            # TB = min(PMx, M+2)  (in place in Hh; Mm dead)
            V.tensor_tensor(Hh3[:, :, 0:512], Hh3[:, :, 0:512], Mm3[:, :, 2:514], op=MIN)
            # B = max(PMn, TB) -> PVn
            V.tensor_tensor(PVn3[:, :, 0:512], PVn3[:, :, 0:512], Hh3[:, :, 0:512], op=MAX)
            # med3(A, B, C): U = min(A,B) -> Hh (TB dead); V2 = max(A,B) -> PA;
            # W2 = min(V2, C) -> PA; OUT = max(U, W2) in place on U in Hh
            V.tensor_tensor(Hh3[:, :, 0:512], PA3[:, :, 0:512], PVn3[:, :, 0:512], op=MIN)
            V.tensor_tensor(PA3[:, :, 0:512], PA3[:, :, 0:512], PVn3[:, :, 0:512], op=MAX)
            V.tensor_tensor(PA3[:, :, 0:512], PA3[:, :, 0:512], PVx3[:, :, 0:512], op=MIN)
            V.tensor_tensor(Hh3[:, :, 0:512], Hh3[:, :, 0:512], PA3[:, :, 0:512], op=MAX)

            # Store: scalar queue normally; last pass splits across both
            # queues to halve the exposed tail.
            STORE_CHUNK = 32  # partitions per store DMA
            last = ps == NPASS - 1
            for ci, p0 in enumerate(range(0, 128, STORE_CHUNK)):
                img = ps * PASS_IMGS + p0 // PIMG
                row0 = (p0 % PIMG) * K
                eng = nc.sync if (last and ci % 2 == 1) else nc.scalar
                eng.dma_start(
                    out=bass.AP(
                        y,
                        img * H * W + row0 * W,
                        [[K * W, STORE_CHUNK], [1, K * W]],
                    ),
                    in_=Hh3[p0 : p0 + STORE_CHUNK, :, 0:512],
                )
    nc.finalize()
    return nc


LAST_EXEC_TIME_NS = None
LAST_TRACE = None


def run(x: np.ndarray, trace: bool = False, offload: bool | None = None):
    """x: (16,3,512,512) fp32 -> (16,3,512,512) fp32 median-blurred."""
    global LAST_EXEC_TIME_NS, LAST_TRACE
    if offload is None:
        offload = OFFLOAD
    assert x.shape == (B, C, H, W), x.shape
    x = np.ascontiguousarray(x, dtype=np.float32)

    key = ("v5", offload)
    if key not in _cache:
        _cache[key] = _build(offload)
    nc = _cache[key]

    xpad = np.pad(x, ((0, 0), (0, 0), (1, 1), (1, 1)))
    shards = xpad.reshape(N_CORES, IMGS, HP, WP)
    in_maps = [{"xp": shards[c]} for c in range(N_CORES)]

    res = run_bass_kernel_spmd(nc, in_maps, list(range(N_CORES)), trace=trace)
    LAST_EXEC_TIME_NS = res.exec_time_ns
    LAST_TRACE = res.instructions_and_trace
    out = np.stack([res.results[c]["y"] for c in range(N_CORES)])
    return np.ascontiguousarray(out.reshape(B, C, H, W))


def kernel(x: np.ndarray) -> np.ndarray:
    return run(x, trace=False)


# revision 20
# speedup vs baseline: 1.2118x; 1.0579x over previous
"""MedianBlur 3x3 (zero-padded) over (16, 3, 512, 512) fp32 on 8 NeuronCores.

Strategy
--------
Pure data parallel: batch dim 16 -> 2 per core; each core processes
6 images (2 batches x 3 channels) of 512x512.

Host side pads each image to 514x514 with zeros, so the device kernel
needs no boundary special-casing: the median of a 3x3 window of the
padded image (windows centered at padded rows/cols 1..512) equals the
reference's zero-padded median exactly.

Device layout: the 6 images are processed in 3 passes of 2 images.
Within a pass, each image occupies 64 partitions; partition p holds
K=8 output rows (10 padded rows with halo, each 514 floats) in its
free dimension, so BOTH the vertical and the horizontal 3-tap window
reads are free-dim offsets -- no transposes, no cross-partition
traffic.

Median-of-9 as a separable min/max network (exact, 18 tensor_tensor
ops per pass):
  vertical sort3 of each column  -> lo (L), mid (M), hi (Hh)
  median9 = med3( max3_h(L), med3_h(M), min3_h(Hh) )

All 18 ops run on VectorE (fp32 tensor_tensor = 1 elem/lane/cycle; the
other engines cannot do 2-input elementwise min/max on this toolchain:
walrus rejects TensorTensor on Pool, ScalarE is unary-only). The
min/max network is the whole compute cost; buffers are reused
aggressively (5 SBUF tiles total) so the OUT staging tile (Hh) can be
double-buffered and stores overlap the next pass.

DMA: each HWDGE engine (sync, scalar) owns ONE ~100 GB/s hardware
queue. All loads are issued up front (X has a fresh slot per pass);
pass 0's load and the last pass's store are split across both engines
to shorten the exposed head/tail. Multi-wait instructions are
legalized by Bacc's generate_event_semaphores (TRN2 instructions
encode at most one sync-wait).
"""

import os
from contextlib import ExitStack

import numpy as np

import concourse.bacc as bacc
import concourse.bass as bass
import concourse.mybir as mybir
import concourse.tile as tile
from concourse.bass_utils import run_bass_kernel_spmd

FP32 = mybir.dt.float32
MIN = mybir.AluOpType.min
MAX = mybir.AluOpType.max

N_CORES = 8
B, C, H, W = 16, 3, 512, 512
IMGS = (B // N_CORES) * C  # images per core = 6
HP, WP = H + 2, W + 2      # zero-padded image
K = 8                      # output rows per partition
XROWS = K + 2              # input rows per partition (halo)
PIMG = H // K              # partitions per image = 64
PASS_IMGS = 128 // PIMG    # images per pass = 2
NPASS = IMGS // PASS_IMGS  # passes = 3

# GPSIMD offload is disabled: this walrus version cannot encode the
# TensorTensor opcode on the Pool engine (ISA check fails at codegen).
OFFLOAD = os.environ.get("MEDIAN_OFFLOAD", "0") == "1"

_cache = {}


def _build(offload: bool):
    # Bacc (not raw Bass): its generate_event_semaphores pass splits
    # multi-wait instructions, which TRN2 hardware cannot encode.
    nc = bacc.Bacc(
        "TRN2", target_bir_lowering=False, debug=False, num_devices=N_CORES
    )
    xp = nc.declare_dram_parameter("xp", [IMGS, HP, WP], FP32, isOutput=False)
    y = nc.declare_dram_parameter("y", [IMGS, H, W], FP32, isOutput=True)

    with ExitStack() as ctx:
        tc = ctx.enter_context(tile.TileContext(nc))
        px = ctx.enter_context(tc.tile_pool(name="px", bufs=4))  # fresh X per pass
        ph = ctx.enter_context(tc.tile_pool(name="ph", bufs=2))
        pt = ctx.enter_context(tc.tile_pool(name="pt", bufs=1))

        V = nc.vector

        # Variable-size passes: small single-image K=4 passes first and
        # last shrink the exposed head (first load) and tail (last store);
        # the middle passes use K=8 with 2 images across 128 partitions.
        PASSES = [(4, 0, 1), (8, 1, 2), (8, 3, 2), (4, 5, 1)]  # (K, img0, n)

        # Issue ALL input loads up front. Each HWDGE engine owns ONE
        # hardware queue; the per-core HBM read floor is ~100 GB/s, so the
        # first pass's load is split across both engines and kept small.
        LOAD_CHUNK = 16  # partitions per load DMA (keeps the queue fed)
        Xs = []
        for ps, (Kp, img0, nimg) in enumerate(PASSES):
            pimg = H // Kp  # partitions per image this pass
            X = px.tile([128, (Kp + 2) * WP], FP32, tag="X")
            Xs.append(X)
            for ci, p0 in enumerate(range(0, 128, LOAD_CHUNK)):
                img = img0 + p0 // pimg
                row0 = (p0 % pimg) * Kp
                eng = nc.scalar if (ps == 0 and ci % 2 == 1) else nc.sync
                eng.dma_start(
                    out=X[p0 : p0 + LOAD_CHUNK, :],
                    in_=bass.AP(
                        xp,
                        img * HP * WP + row0 * WP,
                        [[Kp * WP, LOAD_CHUNK], [1, (Kp + 2) * WP]],
                    ),
                )

        for ps, (Kp, img0, nimg) in enumerate(PASSES):
            K = Kp
            pimg = H // Kp
            X = Xs[ps]
            X3 = X.rearrange("p (r c) -> p r c", c=WP)  # [128, K+2, 514]

            PVn = pt.tile([128, K * WP], FP32, tag="PVn")
            PVx = pt.tile([128, K * WP], FP32, tag="PVx")
            Hh = ph.tile([128, K * WP], FP32, tag="Hh")  # bufs=2: store overlap
            Mm = pt.tile([128, K * WP], FP32, tag="Mm")

            r3 = lambda t: t.rearrange("p (r c) -> p r c", c=WP)
            PVn3, PVx3, Hh3, Mm3 = r3(PVn), r3(PVx), r3(Hh), r3(Mm)
            # PA lives in the X tile (X is dead after the vertical stage)
            PA3 = X3[:, 0:K, :]

            # ---- vertical sort3 (per column), pairwise-shared ----
            V.tensor_tensor(PVn3, X3[:, 0:K, :], X3[:, 1 : K + 1, :], op=MIN)
            V.tensor_tensor(PVx3, X3[:, 0:K, :], X3[:, 1 : K + 1, :], op=MAX)
            # hi = max(pv_max, x+2)
            V.tensor_tensor(Hh3, PVx3, X3[:, 2 : K + 2, :], op=MAX)
            # T = min(pv_max, x+2)   (in place)
            V.tensor_tensor(PVx3, PVx3, X3[:, 2 : K + 2, :], op=MIN)
            # mid = max(pv_min, T)
            V.tensor_tensor(Mm3, PVn3, PVx3, op=MAX)
            # lo = min(pv_min, x+2)  (in place; X dead now)
            V.tensor_tensor(PVn3, PVn3, X3[:, 2 : K + 2, :], op=MIN)
            L3 = PVn3

            # ---- horizontal merge (buffers cycle: every tile all-DVE) ----
            # A = max3_h(L) -> PA (in the dead X tile)
            V.tensor_tensor(PA3[:, :, 0:513], L3[:, :, 0:513], L3[:, :, 1:514], op=MAX)
            V.tensor_tensor(PA3[:, :, 0:512], PA3[:, :, 0:512], L3[:, :, 2:514], op=MAX)
            # C = min3_h(Hh) -> PVx (T dead)
            V.tensor_tensor(PVx3[:, :, 0:513], Hh3[:, :, 0:513], Hh3[:, :, 1:514], op=MIN)
            V.tensor_tensor(PVx3[:, :, 0:512], PVx3[:, :, 0:512], Hh3[:, :, 2:514], op=MIN)
            # mid pairwise: PMn -> PVn (L dead), PMx -> Hh (hi dead)
            V.tensor_tensor(PVn3[:, :, 0:513], Mm3[:, :, 0:513], Mm3[:, :, 1:514], op=MIN)
            V.tensor_tensor(Hh3[:, :, 0:513], Mm3[:, :, 0:513], Mm3[:, :, 1:514], op=MAX)
            # TB = min(PMx, M+2)  (in place in Hh; Mm dead)
            V.tensor_tensor(Hh3[:, :, 0:512], Hh3[:, :, 0:512], Mm3[:, :, 2:514], op=MIN)
            # B = max(PMn, TB) -> PVn
            V.tensor_tensor(PVn3[:, :, 0:512], PVn3[:, :, 0:512], Hh3[:, :, 0:512], op=MAX)
            # med3(A, B, C): U = min(A,B) -> Hh (TB dead); V2 = max(A,B) -> PA;
            # W2 = min(V2, C) -> PA; OUT = max(U, W2) in place on U in Hh
            V.tensor_tensor(Hh3[:, :, 0:512], PA3[:, :, 0:512], PVn3[:, :, 0:512], op=MIN)
            V.tensor_tensor(PA3[:, :, 0:512], PA3[:, :, 0:512], PVn3[:, :, 0:512], op=MAX)
            V.tensor_tensor(PA3[:, :, 0:512], PA3[:, :, 0:512], PVx3[:, :, 0:512], op=MIN)
            V.tensor_tensor(Hh3[:, :, 0:512], Hh3[:, :, 0:512], PA3[:, :, 0:512], op=MAX)

            # Store: scalar queue normally; last pass splits across both
            # queues to halve the exposed tail.
            STORE_CHUNK = 32  # partitions per store DMA
            last = ps == len(PASSES) - 1
            for ci, p0 in enumerate(range(0, 128, STORE_CHUNK)):
                img = img0 + p0 // pimg
                row0 = (p0 % pimg) * K
                eng = nc.sync if (last and ci % 2 == 1) else nc.scalar
                eng.dma_start(
                    out=bass.AP(
                        y,
                        img * H * W + row0 * W,
                        [[K * W, STORE_CHUNK], [1, K * W]],
                    ),
                    in_=Hh3[p0 : p0 + STORE_CHUNK, :, 0:512],
                )
    nc.finalize()
    return nc


LAST_EXEC_TIME_NS = None
LAST_TRACE = None


def run(x: np.ndarray, trace: bool = False, offload: bool | None = None):
    """x: (16,3,512,512) fp32 -> (16,3,512,512) fp32 median-blurred."""
    global LAST_EXEC_TIME_NS, LAST_TRACE
    if offload is None:
        offload = OFFLOAD
    assert x.shape == (B, C, H, W), x.shape
    x = np.ascontiguousarray(x, dtype=np.float32)

    key = ("v6", offload)
    if key not in _cache:
        _cache[key] = _build(offload)
    nc = _cache[key]

    xpad = np.pad(x, ((0, 0), (0, 0), (1, 1), (1, 1)))
    shards = xpad.reshape(N_CORES, IMGS, HP, WP)
    in_maps = [{"xp": shards[c]} for c in range(N_CORES)]

    res = run_bass_kernel_spmd(nc, in_maps, list(range(N_CORES)), trace=trace)
    LAST_EXEC_TIME_NS = res.exec_time_ns
    LAST_TRACE = res.instructions_and_trace
    out = np.stack([res.results[c]["y"] for c in range(N_CORES)])
    return np.ascontiguousarray(out.reshape(B, C, H, W))


def kernel(x: np.ndarray) -> np.ndarray:
    return run(x, trace=False)
